# revision 3
# baseline (speedup 1.0000x reference)
"""Causal self-attention (B=8, T=1024, C=768, H=12) on 8 trn2 NeuronCores.

Strategy: pure data parallelism over the batch dimension — one batch element
per core, no collectives.  Per core:

  1. x^T materialized via PE transposes ([1024,768] -> [768,1024]), pipelined
     offset-by-one with the x-tile DMAs; PE clock gate pre-warmed with dummy
     transposes while the first tile is in flight.  wv streams on the scalar
     HWDGE queue so x tiles are never stuck behind it.
  2. v = x @ w_v computed in [t, c'] layout (lhsT = x^T tiles), stored with an
     interleaved ones-column per head ([t, 12*(64+1)]) so PV matmuls also
     accumulate the softmax denominator (row 64 of the PSUM output).
  3. q^T, k^T computed in [c', t] layout (lhsT = w_attn tiles, rhs = x^T) for
     all 12 M-tiles up front, evicted to bf16 (halves SBUF so all 12 stay
     live, and S^T matmuls take bf16 at full rate).
  4. Attention per head pair (even head at qk-tile partitions 0-63, odd at
     64-127): S^T[tk, tq] = k^T.T @ q^T (K=64 matmuls, both heads writing the
     two halves of one [128,1024] PSUM tile; the two K=64 matmuls execute
     concurrently in different PE row groups — measured 173 ns/pair).  One
     exp per k-tile covers both heads (ScalarE, scale=1/8 folded in; the Act
     engine is per-op issue-bound at ~500 ns so exps are kept maximally wide;
     fully-masked leading columns skipped via a strided AP).  Diagonal blocks
     masked by a 0/1 triangle multiply on DVE.  PV y'^T[65, tq] += v_aug.T @
     P^T accumulates over k-tiles with the rhs/out shrunk to the causally
     live columns (so the masked region is never read — no zero-fills).
     Software pipeline depth 2: PV(i) is emitted after S^T(i+2) (st_psum
     bufs=3, enabled by moving qk production out of the attention loop),
     which gives exp(i) ~770 ns of PE cover — it needs ~700.
  5. Softmax normalization per (head, J): DVE reciprocal of PSUM row 64,
     GpSimd partition_broadcast to 64 partitions, fused normalize+evict
     multiply into y^T (bf16).
  6. out = y @ w_proj + b_proj with lhsT = y^T tiles (bf16 matmuls), output
     in [t, c] layout, contiguous DMA back to DRAM.

Dtypes: x^T/wv/v/P are float32r (full PE rate at moving dim >= 256); q/k and
y^T/w_proj are bf16 (S^T and proj tolerate it; measured rel err ~2e-3 vs the
fp32 reference).  exp OUTPUT stays f32r: bf16 Act writes measured 2.3x
slower (1197 vs 516 ns per [128,1024] exp).
"""

import sys

if "/opt/trn_rl_repo" not in sys.path:
    sys.path.insert(0, "/opt/trn_rl_repo")

from contextlib import ExitStack

import numpy as np

import concourse.bass as bass
import concourse.bacc as bacc
import concourse.mybir as mybir
from concourse import tile
from concourse.masks import make_identity

P = 128
T = 1024
C = 768
H = 12
D = 64
TT = T // P          # 8 t-tiles
KC = C // P          # 6 c-tiles (contraction)
NQK = 2 * C // P     # 12 q/k M-tiles
VW = H * (D + 1)     # 780: v with interleaved ones columns
NEG = -1.0e9

F32 = mybir.dt.float32
F32R = mybir.dt.float32r
BF16 = mybir.dt.bfloat16


def build_nc(mm_dt: str = "f32r", repeat: int = 1, hw_loop: int = 0):
    MDT = {"bf16": BF16, "f32r": F32R, "f32": F32}[mm_dt]  # x^T / v / P dtype
    QDT = BF16                                             # q/k tiles
    YDT = BF16                                             # y^T / w_proj

    nc = bacc.Bacc(None)
    x_d = nc.declare_dram_parameter("x", [T, C], F32, isOutput=False)
    wa_d = nc.declare_dram_parameter("w_attn", [C, 3 * C], MDT, isOutput=False)
    ba_d = nc.declare_dram_parameter("b_attn", [3 * C], F32, isOutput=False)
    wp_d = nc.declare_dram_parameter("w_proj", [C, C], YDT, isOutput=False)
    bp_d = nc.declare_dram_parameter("b_proj", [C], F32, isOutput=False)
    out_d = nc.declare_dram_parameter("out", [T, C], F32, isOutput=True)

    with tile.TileContext(nc) as tc, ExitStack() as ctx:
        const = ctx.enter_context(tc.tile_pool(name="const", bufs=1))
        identity = const.tile([P, P], F32)
        make_identity(nc, identity)
        # 0/1 triangle mask for diagonal blocks: cm01[p,c] = 1 if c >= p else 0
        cm01 = const.tile([P, P], F32)
        nc.gpsimd.memset(cm01[:], 1.0)
        nc.gpsimd.affine_select(
            out=cm01[:],
            in_=cm01[:],
            compare_op=mybir.AluOpType.is_ge,
            fill=0.0,
            base=0,
            pattern=[[1, P]],
            channel_multiplier=-1,
        )
        ba_cols = const.tile([P, NQK], F32)
        bav = const.tile([P, C], F32)
        bpb = const.tile([P, C], F32)

        persist = ctx.enter_context(tc.tile_pool(name="persist", bufs=1))
        xT = persist.tile([P, KC, T], MDT)      # x^T: [c%128, c//128, t]
        wv = persist.tile([P, KC, C], MDT)      # w_attn[:, 2C:3C]
        wp = persist.tile([P, KC, C], YDT)      # w_proj (bf16)
        v_all = persist.tile([P, TT, VW], MDT)  # v + ones cols
        yT = persist.tile([P, KC, T], YDT)      # y^T (normalized, bf16)
        qk = persist.tile([P, NQK, T], QDT)     # all 12 q^T/k^T tiles (bf16)
        # pre-fill with 1.0: v evictions overwrite the data columns, leaving
        # the interleaved per-head ones-columns at 1.0
        nc.gpsimd.memset(v_all[:] if MDT != F32R else v_all[:].bitcast(F32), 1.0)

        # x tiles stream alone on the sync queue; everything else
        # (wv/biases/wp) goes on the scalar HWDGE queue.
        wa_v = wa_d[:, 2 * C : 3 * C].rearrange("(a p) n -> p a n", p=P)
        nc.scalar.dma_start(wv[:, :, :384], wa_v[:, :, :384])
        nc.scalar.dma_start(wv[:, :, 384:], wa_v[:, :, 384:])
        nc.scalar.dma_start(bav[:], ba_d[2 * C : 3 * C][None, :].to_broadcast((P, C)))
        nc.scalar.dma_start(bpb[:], bp_d[:][None, :].to_broadcast((P, C)))
        nc.scalar.dma_start(
            ba_cols[:], ba_d[: 2 * C].rearrange("(a p) -> p a", p=P)
        )

        xpool = ctx.enter_context(tc.tile_pool(name="xpool", bufs=3))
        # one shared PSUM pool: 3 slots x [128,1024] (6 banks) serve the
        # [128,512] mm tiles of phases 1/2/4 AND the attention S^T tiles
        pspool = ctx.enter_context(tc.tile_pool(name="pspool", bufs=3, space="PSUM"))

        wapool = ctx.enter_context(tc.tile_pool(name="wapool", bufs=3))
        y_psum = ctx.enter_context(tc.tile_pool(name="y_psum", bufs=2, space="PSUM"))
        ptpool = ctx.enter_context(tc.tile_pool(name="ptpool", bufs=4))
        zpool = ctx.enter_context(tc.tile_pool(name="zpool", bufs=2))
        zrpool = ctx.enter_context(tc.tile_pool(name="zrpool", bufs=2))
        outpool = ctx.enter_context(tc.tile_pool(name="outpool", bufs=2))
        import contextlib

        loop_cm = (
            tc.For_i(
                0,
                hw_loop,
                1,
                hint_engines=(
                    mybir.EngineType.PE,
                    mybir.EngineType.DVE,
                    mybir.EngineType.Activation,
                    mybir.EngineType.SP,
                    mybir.EngineType.Pool,
                ),
            )
            if hw_loop
            else contextlib.nullcontext()
        )
        with loop_cm:
            for _rep in range(repeat):
                # warm the PE clock gate (HAM) while the first x tile is in flight
                warm_ps = pspool.tile([P, 512], F32, tag="ps", name="warm")
                for _ in range(10):
                    nc.tensor.transpose(warm_ps[:, :P], identity[:], identity[:])

                # ---- phase 1: transpose x, compute v (offset-by-one pipeline) ----
                xts = {}

                def load_x(tt):
                    xt = xpool.tile([P, C], F32, tag="x", name="xt")
                    nc.sync.dma_start(xt[:], x_d[tt * P : (tt + 1) * P, :])
                    xts[tt] = xt

                def trans_x(tt):
                    xt = xts.pop(tt)
                    for grp in range(2):
                        pst = pspool.tile([P, 512], F32, tag="ps", name="tps")
                        for j in range(3):
                            kc = 3 * grp + j
                            nc.tensor.transpose(
                                pst[:, j * P : (j + 1) * P], xt[:, kc * P : (kc + 1) * P], identity
                            )
                        nc.vector.tensor_copy(
                            xT[:, 3 * grp : 3 * grp + 3, tt * P : (tt + 1) * P],
                            pst[:, :384].rearrange("p (a b) -> p a b", b=P),
                        )

                def v_mm(tt, nn):
                    pst = pspool.tile([P, 512], F32, tag="ps", name="vps")
                    ps = pst[:, :384]
                    for kc in range(KC):
                        nc.tensor.matmul(
                            ps,
                            xT[:, kc, tt * P : (tt + 1) * P],
                            wv[:, kc, nn * 384 : (nn + 1) * 384],
                            start=(kc == 0),
                            stop=(kc == KC - 1),
                        )
                    vview = v_all[:, tt, :].rearrange("p (h e) -> p h e", e=D + 1)[
                        :, nn * 6 : (nn + 1) * 6, :D
                    ]
                    nc.vector.tensor_add(
                        vview,
                        ps.rearrange("p (h e) -> p h e", e=D),
                        bav[:, nn * 384 : (nn + 1) * 384].rearrange("p (h e) -> p h e", e=D),
                    )

                load_x(0)
                load_x(1)
                trans_x(0)
                load_x(2)
                trans_x(1)
                for tt in range(2, TT):
                    if tt + 1 < TT:
                        load_x(tt + 1)
                    trans_x(tt)
                    v_mm(tt - 2, 0)
                    v_mm(tt - 2, 1)
                for tt in range(TT - 2, TT):
                    v_mm(tt, 0)
                    v_mm(tt, 1)

                # ---- phase 2: all q^T/k^T M-tiles up front (bf16 eviction) ----

                wa_r = wa_d[:, :].rearrange("(a p) n -> p a n", p=P)
                nc.scalar.dma_start(wp[:], wp_d[:, :].rearrange("(a p) n -> p a n", p=P))

                def emit_qk(m):
                    wt = wapool.tile([P, KC, P], MDT, tag="wa", name="wt")
                    nc.sync.dma_start(wt[:], wa_r[:, :, m * P : (m + 1) * P])
                    for nn in range(2):
                        ps = pspool.tile([P, 512], F32, tag="ps", name="qps")
                        for kc in range(KC):
                            nc.tensor.matmul(
                                ps,
                                wt[:, kc, :],
                                xT[:, kc, nn * 512 : (nn + 1) * 512],
                                start=(kc == 0),
                                stop=(kc == KC - 1),
                            )
                        nc.vector.tensor_scalar_add(
                            qk[:, m, nn * 512 : (nn + 1) * 512], ps, ba_cols[:, m : m + 1]
                        )

                for m in range(NQK):
                    emit_qk(m)

                # ---- phase 3: attention, deep-pipelined S -> exp -> PV ----
                for pr in range(6):
                    # Head pair processed together: head A (even) occupies
                    # qk-tile partitions 0-63, head B (odd) partitions 64-127.
                    hA, hB = 2 * pr, 2 * pr + 1
                    for J in range(2):
                        nk = 4 * J + 4
                        ypA = y_psum.tile([D + 1, 512], F32, tag="y", name="ypA")
                        ypB = y_psum.tile([D + 1, 512], F32, tag="y", name="ypB")
                        pts = {}

                        def emit_s(i, pr=pr, J=J):
                            st = pspool.tile([P, 1024], F32, tag="ps", name="st")
                            for s in range(2):
                                par = D * s
                                nc.tensor.matmul(
                                    st[:, s * 512 : (s + 1) * 512],
                                    qk[par : par + D, 6 + pr, i * P : (i + 1) * P],
                                    qk[par : par + D, pr, J * 512 : (J + 1) * 512],
                                    start=True,
                                    stop=True,
                                )
                            pt = ptpool.tile([P, 1024], MDT, tag="pt")
                            pts[i] = pt
                            jj = i - 4 * J
                            if jj > 0:
                                # skip the fully-masked first 128*jj columns of
                                # both halves (never read downstream)
                                w0 = P * jj
                                st2 = st[:, :].rearrange("p (s c) -> p s c", s=2)
                                pt2 = pt[:, :].rearrange("p (s c) -> p s c", s=2)
                                nc.scalar.activation(
                                    pt2[:, :, w0:],
                                    st2[:, :, w0:],
                                    mybir.ActivationFunctionType.Exp,
                                    scale=0.125,
                                )
                            else:
                                nc.scalar.activation(
                                    pt[:],
                                    st[:],
                                    mybir.ActivationFunctionType.Exp,
                                    scale=0.125,
                                )
                            if jj >= 0:
                                # zero the upper-triangular part of the diagonal block
                                blk = pt[:, :].rearrange("p (s c) -> p s c", s=2)[
                                    :, :, P * jj : P * (jj + 1)
                                ]
                                nc.vector.tensor_mul(
                                    blk,
                                    blk if MDT != F32R else blk.bitcast(F32),
                                    cm01[:, None, :].to_broadcast((P, 2, P)),
                                )

                        def emit_pv(i, J=J, ypA=ypA, ypB=ypB):
                            ptp = pts.pop(i)
                            jj = i - 4 * J
                            w0 = P * jj if jj > 0 else 0
                            for s, hh, yp in ((0, hA, ypA), (1, hB, ypB)):
                                nc.tensor.matmul(
                                    yp[:, w0:] if w0 else yp[:, :],
                                    v_all[:, i, (D + 1) * hh : (D + 1) * (hh + 1)],
                                    ptp[:, s * 512 + w0 : (s + 1) * 512],
                                    start=(i == 0),
                                    stop=(i == nk - 1),
                                    skip_group_check=True,
                                )

                        # pipeline: PV(i) lands after S(i+2) so exp(i) has
                        # ~770 ns of PE cover
                        for i in range(nk):
                            emit_s(i)
                            if i >= 2:
                                emit_pv(i - 2)
                        emit_pv(nk - 2)
                        emit_pv(nk - 1)

                        for hh, yp in ((hA, ypA), (hB, ypB)):
                            zr = zpool.tile([1, 512], F32, tag="z")
                            nc.vector.reciprocal(zr[0:1, :], yp[D : D + 1, :])
                            zrep = zrpool.tile([D, 512], F32, tag="zr")
                            nc.gpsimd.partition_broadcast(zrep[:], zr[0:1, :])
                            kc_y = hh // 2
                            par_y = D * (hh % 2)
                            nc.vector.tensor_mul(
                                yT[par_y : par_y + D, kc_y, J * 512 : (J + 1) * 512],
                                yp[:D, :],
                                zrep[:],
                            )

                # ---- phase 4: output projection (bf16) ----
                for tt in range(TT):
                    ot = outpool.tile([P, C], F32, tag="out")
                    for nn in range(2):
                        pst = pspool.tile([P, 512], F32, tag="ps", name="pps")
                        ps = pst[:, :384]
                        for kc in range(KC):
                            nc.tensor.matmul(
                                ps,
                                yT[:, kc, tt * P : (tt + 1) * P],
                                wp[:, kc, nn * 384 : (nn + 1) * 384],
                                start=(kc == 0),
                                stop=(kc == KC - 1),
                            )
                        nc.vector.tensor_add(
                            ot[:, nn * 384 : (nn + 1) * 384], ps, bpb[:, nn * 384 : (nn + 1) * 384]
                        )
                        nc.sync.dma_start(
                            out_d[tt * P : (tt + 1) * P, nn * 384 : (nn + 1) * 384],
                            ot[:, nn * 384 : (nn + 1) * 384],
                        )

    nc.finalize()
    return nc


_cache = {}
MM_DT = "f32r"


def get_nc():
    if "nc" not in _cache:
        _cache["nc"] = build_nc(mm_dt=MM_DT)
    return _cache["nc"]


def kernel(x, w_attn, b_attn, w_proj, b_proj):
    import ml_dtypes

    wdt = ml_dtypes.bfloat16 if MM_DT == "bf16" else np.float32
    x = np.ascontiguousarray(np.asarray(x, dtype=np.float32))
    w_attn = np.ascontiguousarray(np.asarray(w_attn, dtype=np.float32).astype(wdt))
    b_attn = np.ascontiguousarray(np.asarray(b_attn, dtype=np.float32))
    w_proj = np.ascontiguousarray(
        np.asarray(w_proj, dtype=np.float32).astype(ml_dtypes.bfloat16)
    )
    b_proj = np.ascontiguousarray(np.asarray(b_proj, dtype=np.float32))

    from concourse.bass_utils import run_bass_kernel_spmd

    nc = get_nc()
    B = x.shape[0]
    assert B == 8
    in_maps = [
        dict(
            x=np.ascontiguousarray(x[b]),
            w_attn=w_attn,
            b_attn=b_attn,
            w_proj=w_proj,
            b_proj=b_proj,
        )
        for b in range(B)
    ]
    res = run_bass_kernel_spmd(nc, in_maps, list(range(B))).results
    return np.stack([res[b]["out"] for b in range(B)], axis=0)


if __name__ == "__main__":
    x = np.random.randn(8, T, C).astype(np.float32)
    w_attn = (np.random.randn(C, 3 * C) * 0.02).astype(np.float32)
    b_attn = np.zeros(3 * C, np.float32)
    w_proj = (np.random.randn(C, C) * 0.02).astype(np.float32)
    b_proj = np.zeros(C, np.float32)
    y = kernel(x, w_attn, b_attn, w_proj, b_proj)
    print(y.shape, y.dtype)
